# revision 4
# baseline (speedup 1.0000x reference)
"""LIF (leaky integrate-and-fire) forward kernel for Trainium2, 8 NeuronCores.

Reference semantics (per element, scan over T):
    u = LAM * u + x_t
    o_t = (u - THRESHOLD > 0) ? 1.0 : 0.0
    u = u - o_t

Sharding: pure data parallel over batch B=16 -> 2 samples per core.
Per-core: C=128 channels on SBUF partitions. Host pre-transposes each
shard to t-major [T, B_LOC, C, HW] so (t, b) merge into one AP dim ->
3-dim DMA access patterns, 4 MiB loads / 2 MiB rolling stores.

Compute is split into two independent per-sample chains so the DVE and
GPSIMD pipeline across the sequential T recurrence:
    DVE:    u = (u * LAM) + x_t   (scalar_tensor_tensor, both chains)
            u = u - o_t           (tensor_tensor, both chains)
    GPSIMD: o_t = (u > THRESHOLD) (tensor_scalar is_gt, both chains)
All arithmetic is IEEE fp32 -> bit-identical to the jax CPU reference
(mul-by-0.5 exact; add/sub correctly rounded; compare exact).
"""

import numpy as np

B, T, C, HW = 16, 16, 128, 1024  # HW = 32*32
N_CORES = 8
B_LOC = B // N_CORES  # 2
CHUNK = 4  # timesteps per input DMA chunk (4 MiB)
LAM = 0.5
THRESHOLD = 1.0

_CACHE = {}


def _build():
    import concourse.tile as tile
    import concourse.mybir as mybir
    from concourse import bacc

    nc = bacc.Bacc(
        "TRN2",
        target_bir_lowering=False,
        debug=False,
        enable_asserts=False,
        num_devices=N_CORES,
    )
    f32 = mybir.dt.float32
    Alu = mybir.AluOpType

    x_dram = nc.dram_tensor("x", [T, B_LOC, C, HW], f32, kind="ExternalInput")
    o_dram = nc.dram_tensor("o", [T, B_LOC, C, HW], f32, kind="ExternalOutput")
    # partition dim = C; (t, b) merge into one stride-131072 dim
    x = x_dram.ap().rearrange("t b c f -> c (t b) f")
    o = o_dram.ap().rearrange("t b c f -> c (t b) f")

    with tile.TileContext(nc) as tc:
        with (
            tc.tile_pool(name="xp", bufs=2) as xp,
            tc.tile_pool(name="op", bufs=3) as op_,
            tc.tile_pool(name="up", bufs=1) as up,
        ):
            uA = up.tile([C, HW], f32, name="uA")
            uB = up.tile([C, HW], f32, name="uB")
            nc.vector.memset(uA[:], 0.0)
            nc.vector.memset(uB[:], 0.0)
            for t0 in range(0, T, CHUNK):
                xt = xp.tile([C, CHUNK * B_LOC, HW], f32)
                lo = t0 * B_LOC
                if t0 == 0:
                    # split first load so compute starts sooner
                    nc.sync.dma_start(xt[:, 0:2], x[:, 0:2])
                    nc.sync.dma_start(xt[:, 2:8], x[:, 2:8])
                else:
                    nc.sync.dma_start(xt[:], x[:, lo : lo + CHUNK * B_LOC])
                ot = op_.tile([C, CHUNK * B_LOC, HW], f32)
                for ti in range(CHUNK):
                    xA, xB = xt[:, 2 * ti], xt[:, 2 * ti + 1]
                    oA, oB = ot[:, 2 * ti], ot[:, 2 * ti + 1]
                    # leaky integrate: u = (u * LAM) + x_t
                    nc.vector.scalar_tensor_tensor(
                        uA[:], uA[:], LAM, xA, op0=Alu.mult, op1=Alu.add
                    )
                    # spike: o = (u > 1) as 1.0/0.0  (GPSIMD, off DVE)
                    nc.gpsimd.tensor_scalar(
                        oA, uA[:], THRESHOLD, None, op0=Alu.is_gt
                    )
                    nc.vector.scalar_tensor_tensor(
                        uB[:], uB[:], LAM, xB, op0=Alu.mult, op1=Alu.add
                    )
                    nc.gpsimd.tensor_scalar(
                        oB, uB[:], THRESHOLD, None, op0=Alu.is_gt
                    )
                    # soft reset: u = u - o
                    nc.vector.tensor_tensor(uA[:], uA[:], oA, op=Alu.subtract)
                    nc.vector.tensor_tensor(uB[:], uB[:], oB, op=Alu.subtract)
                    if ti % 2 == 1:
                        # rolling 2-step (2 MiB) output store
                        nc.sync.dma_start(
                            o[:, lo + (ti - 1) * B_LOC : lo + (ti + 1) * B_LOC],
                            ot[:, (ti - 1) * B_LOC : (ti + 1) * B_LOC],
                        )
    nc.compile()
    return nc


def _get_nc():
    if "nc" not in _CACHE:
        _CACHE["nc"] = _build()
    return _CACHE["nc"]


def kernel(x_seq, noise=None, **_ignored):
    from concourse import bass_utils

    nc = _get_nc()
    x = np.ascontiguousarray(np.asarray(x_seq), dtype=np.float32).reshape(
        B, T, C, HW
    )
    in_maps = []
    for i in range(N_CORES):
        shard = x[i * B_LOC : (i + 1) * B_LOC]  # [B_LOC, T, C, HW]
        shard_t = np.ascontiguousarray(shard.transpose(1, 0, 2, 3))
        in_maps.append({"x": shard_t})
    res = bass_utils.run_bass_kernel_spmd(
        nc, in_maps, core_ids=list(range(N_CORES))
    )
    outs = []
    for r in res.results:
        # [T, B_LOC, C, HW] -> [B_LOC, T, C, HW]
        outs.append(np.asarray(r["o"]).transpose(1, 0, 2, 3))
    out = np.concatenate(outs, axis=0)
    return out.reshape(B, T, C, 32, 32)


# revision 5
# speedup vs baseline: 4.7427x; 4.7427x over previous
"""LIF (leaky integrate-and-fire) forward kernel for Trainium2, 8 NeuronCores.

Reference semantics (per element, scan over T):
    u = LAM * u + x_t
    o_t = (u - THRESHOLD > 0) ? 1.0 : 0.0
    u = u - o_t

Sharding: pure data parallel over batch B=16 -> 2 samples per core.
Per-core: C=128 channels on SBUF partitions. Host pre-transposes each
shard to t-major [T, B_LOC, C, HW] so (t, b) merge into one AP dim ->
3-dim DMA access patterns, 4 MiB loads / 2 MiB rolling stores.

Compute is split into two independent per-sample chains so the DVE and
GPSIMD pipeline across the sequential T recurrence:
    DVE:    u = (u * LAM) + x_t   (scalar_tensor_tensor, both chains)
            u = u - o_t           (tensor_tensor, both chains)
    GPSIMD: o_t = (u > THRESHOLD) (tensor_scalar is_gt, both chains)
All arithmetic is IEEE fp32 -> bit-identical to the jax CPU reference
(mul-by-0.5 exact; add/sub correctly rounded; compare exact).
"""

import numpy as np

B, T, C, HW = 16, 16, 128, 1024  # HW = 32*32
N_CORES = 8
B_LOC = B // N_CORES  # 2
CHUNK = 4  # timesteps per input DMA chunk (4 MiB)
LAM = 0.5
THRESHOLD = 1.0

_CACHE = {}


def _build():
    import concourse.tile as tile
    import concourse.mybir as mybir
    from concourse import bacc

    nc = bacc.Bacc(
        "TRN2",
        target_bir_lowering=False,
        debug=False,
        enable_asserts=False,
        num_devices=N_CORES,
    )
    f32 = mybir.dt.float32
    Alu = mybir.AluOpType

    x_dram = nc.dram_tensor("x", [T, B_LOC, C, HW], f32, kind="ExternalInput")
    o_dram = nc.dram_tensor("o", [T, B_LOC, C, HW], f32, kind="ExternalOutput")
    # partition dim = C; (t, b) merge into one stride-131072 dim
    x = x_dram.ap().rearrange("t b c f -> c (t b) f")
    o = o_dram.ap().rearrange("t b c f -> c (t b) f")

    with tile.TileContext(nc) as tc:
        with (
            tc.tile_pool(name="xp", bufs=2) as xp,
            tc.tile_pool(name="op", bufs=3) as op_,
            tc.tile_pool(name="up", bufs=1) as up,
        ):
            u = up.tile([C, B_LOC, HW], f32, name="u")
            nc.vector.memset(u[:], 0.0)
            for t0 in range(0, T, CHUNK):
                xt = xp.tile([C, CHUNK * B_LOC, HW], f32)
                lo = t0 * B_LOC
                if t0 == 0:
                    # split first load so compute starts sooner
                    nc.sync.dma_start(xt[:, 0:2], x[:, 0:2])
                    nc.sync.dma_start(xt[:, 2:8], x[:, 2:8])
                else:
                    nc.sync.dma_start(xt[:], x[:, lo : lo + CHUNK * B_LOC])
                ot = op_.tile([C, CHUNK * B_LOC, HW], f32)
                for ti in range(CHUNK):
                    xs = xt[:, 2 * ti : 2 * ti + 2]  # [C, B_LOC, HW]
                    os_ = ot[:, 2 * ti : 2 * ti + 2]
                    # leaky integrate: u = (u * LAM) + x_t
                    nc.vector.scalar_tensor_tensor(
                        u[:], u[:], LAM, xs, op0=Alu.mult, op1=Alu.add
                    )
                    # spike: o = (u > 1) as 1.0/0.0
                    nc.vector.tensor_scalar(
                        os_, u[:], THRESHOLD, None, op0=Alu.is_gt
                    )
                    # soft reset: u = u - o
                    nc.vector.tensor_tensor(u[:], u[:], os_, op=Alu.subtract)
                    if ti % 2 == 1:
                        # rolling 2-step (2 MiB) output store
                        nc.sync.dma_start(
                            o[:, lo + (ti - 1) * B_LOC : lo + (ti + 1) * B_LOC],
                            ot[:, (ti - 1) * B_LOC : (ti + 1) * B_LOC],
                        )
    nc.compile()
    return nc


def _get_nc():
    if "nc" not in _CACHE:
        _CACHE["nc"] = _build()
    return _CACHE["nc"]


def kernel(x_seq, noise=None, **_ignored):
    from concourse import bass_utils

    nc = _get_nc()
    x = np.ascontiguousarray(np.asarray(x_seq), dtype=np.float32).reshape(
        B, T, C, HW
    )
    in_maps = []
    for i in range(N_CORES):
        shard = x[i * B_LOC : (i + 1) * B_LOC]  # [B_LOC, T, C, HW]
        shard_t = np.ascontiguousarray(shard.transpose(1, 0, 2, 3))
        in_maps.append({"x": shard_t})
    res = bass_utils.run_bass_kernel_spmd(
        nc, in_maps, core_ids=list(range(N_CORES))
    )
    outs = []
    for r in res.results:
        # [T, B_LOC, C, HW] -> [B_LOC, T, C, HW]
        outs.append(np.asarray(r["o"]).transpose(1, 0, 2, 3))
    out = np.concatenate(outs, axis=0)
    return out.reshape(B, T, C, 32, 32)
